# revision 18
# baseline (speedup 1.0000x reference)
"""Trainium2 Bass kernel for nn_MultiHeadAttention_3539053052118.

GQA attention (B=2, S=2048, HID=2048, 16 q-heads, 4 kv-heads, RoPE, causal)
distributed over 8 NeuronCores: 2-way data-parallel over batch x 4-way
tensor-parallel over kv-head groups. Each core computes q/kv projections for
its 4 q-heads + 1 kv-head (bf16 matmuls; inputs are pre-converted to bf16 and
pre-laid-out on the host so no on-chip casts are needed), RoPE, causal flash
attention with a globally software-pipelined scores->exp->sums/ctx chain;
each head's context is AllGather-ed (bf16) within the 4-core batch group as
soon as it is ready (collective input DMAs ride the otherwise-idle GpSimd
queue so they are never stuck behind bulk loads), and the o_proj accumulates
per-wave into SBUF so the collectives overlap attention. Each core produces a
distinct 512-column slice of the output.
"""

import math
import sys
import types

sys.path.insert(0, "/opt/trn_rl_repo")

import antenv  # noqa: F401

if "antenv.axon_hooks" not in sys.modules:
    _hooks = types.ModuleType("antenv.axon_hooks")
    _hook_box = {"hook": None}
    _hooks.set_axon_ntff_profile_hook = lambda h: _hook_box.__setitem__("hook", h)
    _hooks.get_axon_ntff_profile_hook = lambda: _hook_box["hook"]
    sys.modules["antenv.axon_hooks"] = _hooks
    try:
        from trn_agent_boot.trn_boot import _ntff_profile_via_ctypes

        _hooks.set_axon_ntff_profile_hook(
            _ntff_profile_via_ctypes("/opt/axon/libaxon_pjrt.so")
        )
    except Exception:
        pass

import numpy as np
import ml_dtypes
import concourse.bass as bass
import concourse.mybir as mybir
import concourse.tile as tile
from concourse import bacc
from concourse import bass_utils
from concourse.masks import make_identity

F32 = mybir.dt.float32
F32R = mybir.dt.float32r
BF16 = mybir.dt.bfloat16
I32 = mybir.dt.int32
AF = mybir.ActivationFunctionType
ALU = mybir.AluOpType

B, S, HID = 2, 2048, 2048
NH, NKV = 16, 4
HD = 128
ROPE_BASE = 10000.0
PI = math.pi

N_CORES = 8
TP = 4
HG = NH // TP  # 4 q heads per core
GROUPS = [[0, 1, 2, 3], [4, 5, 6, 7]]

NKC = HID // 128  # 16 contraction tiles
NQC = S // 512  # 4 q/n chunks
NST = S // 128  # 16 s tiles
OC = 512  # output columns per core

_CACHE = {}


def _build():
    nc = bacc.Bacc("TRN2", target_bir_lowering=False, debug=False, num_devices=N_CORES)

    # host-prepared bf16 inputs, already in [partition, ktile, free] layout
    xh = nc.dram_tensor("xh", [128, NKC, S], BF16, kind="ExternalInput").ap()
    wqh = nc.dram_tensor("wqh", [128, NKC, HG * HD], BF16, kind="ExternalInput").ap()
    wkh = nc.dram_tensor("wkh", [128, NKC, HD], BF16, kind="ExternalInput").ap()
    wvh = nc.dram_tensor("wvh", [128, NKC, HD], BF16, kind="ExternalInput").ap()
    woh = nc.dram_tensor("woh", [128, NKC, OC], BF16, kind="ExternalInput").ap()
    # rope tables, computed on host from position_ids (valid for arbitrary
    # positions; avoids the on-device sin-range reduction chain entirely)
    sinh = nc.dram_tensor("sinh", [HD, S], BF16, kind="ExternalInput").ap()
    cosh = nc.dram_tensor("cosh", [HD, S], BF16, kind="ExternalInput").ap()
    out = nc.dram_tensor("out_slice", [S, OC], F32, kind="ExternalOutput").ap()

    # per-head collective bounce buffers (separate tensors so AG(h) only
    # depends on head h's writes)
    cc_in = [
        [nc.dram_tensor(f"cc_in{h}_{hf}", [HD, S // 2], BF16).ap() for hf in range(2)]
        for h in range(HG)
    ]
    cc_out = [
        [
            nc.dram_tensor(f"cc_out{h}_{hf}", [TP * HD, S // 2], BF16).ap()
            for hf in range(2)
        ]
        for h in range(HG)
    ]
    # quarter-granularity buffers for the very last gathers (tail latency)
    cc_in_q = [nc.dram_tensor(f"cc_inq{j}", [HD, 512], BF16).ap() for j in range(2)]
    cc_out_q = [
        nc.dram_tensor(f"cc_outq{j}", [TP * HD, 512], BF16).ap() for j in range(2)
    ]
    # tiny warm-up collective: absorbs first-collective setup cost and
    # re-syncs the cores right at kernel start
    cc_wout = nc.dram_tensor("cc_wout", [TP, 64], BF16).ap()

    # ---- inline constants ----
    half = HD // 2
    R = np.zeros((HD, HD), np.float32)
    for p in range(half):
        R[p, p + half] = -1.0
    for p in range(half, HD):
        R[p, p - half] = 1.0
    permRT_c = nc.inline_tensor(
        np.ascontiguousarray(R.T).astype(ml_dtypes.bfloat16), "permRT"
    ).ap()
    ones_row_bf_c = nc.inline_tensor(
        np.ones((1, 128), ml_dtypes.bfloat16), "ones_row_bf"
    ).ap()
    ones_col_c = nc.inline_tensor(
        np.ones((128, 1), ml_dtypes.bfloat16), "ones_col"
    ).ap()
    # causal mask: M[p, j] = 0 where key p > query j (within diag subtile)
    mtri = np.where(
        np.arange(128)[:, None] > np.arange(128)[None, :], 0.0, 1.0
    ).astype(ml_dtypes.bfloat16)
    mtri_c = nc.inline_tensor(mtri, "mtri").ap()
    ident_bf_c = nc.inline_tensor(
        np.eye(128, dtype=ml_dtypes.bfloat16), "ident_bf"
    ).ap()
    warm_c = nc.inline_tensor(np.ones((1, 64), ml_dtypes.bfloat16), "warm").ap()

    with tile.TileContext(nc) as tc:
        with (
            tc.tile_pool(name="const", bufs=1) as cpool,
            tc.tile_pool(name="w", bufs=1) as wpool,
            tc.tile_pool(name="attn", bufs=2) as apool,
        ):
            qkvpool = tc.alloc_tile_pool(name="qkv", bufs=1)
            # ---- constants ----
            permRT_sb = cpool.tile([HD, HD], BF16)
            nc.scalar.dma_start(out=permRT_sb[:, :], in_=permRT_c[:, :])
            ones_row_bf = cpool.tile([1, 128], BF16)
            nc.scalar.dma_start(out=ones_row_bf[:, :], in_=ones_row_bf_c[:, :])
            ones_col_sb = cpool.tile([128, 1], BF16)
            nc.scalar.dma_start(out=ones_col_sb[:, :], in_=ones_col_c[:, :])
            mtri_sb = cpool.tile([128, 128], BF16)
            nc.scalar.dma_start(out=mtri_sb[:, :], in_=mtri_c[:, :])
            ident_bf = cpool.tile([128, 128], BF16)
            nc.scalar.dma_start(out=ident_bf[:, :], in_=ident_bf_c[:, :])

            # warm-up AllGather: first in the CC queue, runs during phase 0/1
            nc.gpsimd.collective_compute(
                "AllGather",
                mybir.AluOpType.bypass,
                replica_groups=GROUPS,
                ins=[warm_c[:, :]],
                outs=[cc_wout[:, :]],
            )

            # host-computed rope tables (loaded after the chunk-0 operands,
            # see below — they are not needed until the first rope)
            sinT = cpool.tile([128, S], BF16, tag="tab_sin", name="tab_sin")
            cosT = cpool.tile([128, S], BF16, tag="tab_cos", name="tab_cos")

            # ---- persistent weights (bf16, direct DMA, no casts) ----
            wq_sb = wpool.tile([128, NKC, HG * HD], BF16, tag="wq", name="wq_sb")
            wk_sb = wpool.tile([128, NKC, HD], BF16, tag="wk", name="wk_sb")
            wv_sb = wpool.tile([128, NKC, HD], BF16, tag="wv", name="wv_sb")
            wo_sb = wpool.tile([128, NKC, OC], BF16, tag="wo", name="wo_sb")
            # persistent qkv storage (bf16)
            q_sb = [
                qkvpool.tile([128, S], BF16, tag=f"q{h}", name=f"q{h}")
                for h in range(HG)
            ]
            k_sb = qkvpool.tile([128, S], BF16, tag="k", name="k_sb")
            vT_sb = qkvpool.tile([128, S], BF16, tag="vT", name="vT_sb")
            v_sb = [
                qkvpool.tile([128, HD], BF16, tag=f"v{i}", name=f"v{i}")
                for i in range(NST)
            ]

            xspool = tc.alloc_tile_pool(name="xs", bufs=2)
            psA = tc.alloc_tile_pool(name="psA", bufs=1, space="PSUM")
            psB = tc.alloc_tile_pool(name="psB", bufs=1, space="PSUM")
            psR = tc.alloc_tile_pool(name="psR", bufs=1, space="PSUM")

            # interleave chunk-0 x with weights so the kt=0 operands land
            # first; everything is already bf16 so DMAs feed matmuls directly.
            # wk/wv are only needed by sub-wave B (after all 16 kts of A) and
            # the rope tables only at the first rope, so they load after.
            x_sb = [None] * NQC
            x_sb[0] = xspool.tile([128, NKC, 512], BF16, tag="x", name="x_0")
            for j in range(4):
                ks = slice(j * 4, (j + 1) * 4)
                nc.sync.dma_start(out=x_sb[0][:, ks, :], in_=xh[:, ks, 0:512])
                nc.sync.dma_start(out=wq_sb[:, ks, :], in_=wqh[:, ks, :])
            nc.sync.dma_start(out=wk_sb[:, :, :], in_=wkh[:, :, :])
            nc.sync.dma_start(out=wv_sb[:, :, :], in_=wvh[:, :, :])
            nc.sync.dma_start(out=sinT[:, :], in_=sinh[:, :])
            nc.sync.dma_start(out=cosT[:, :], in_=cosh[:, :])

            # ---- phase 1: projections + rope + v transpose ----
            # rope/v-transpose of chunk q is deferred and interleaved into the
            # PE stream of chunk q+1 (or early attention) so the PE never
            # waits head-of-line on the DVE rope chain.
            pending_items = []  # closures emitting one deferred PE item each
            psO_box = [None]  # filled once the attention-phase psO pool exists

            def emit_rope(qq, idx, pool=None, tag="rot"):
                ns_ = slice(qq * 512, (qq + 1) * 512)
                tgt = q_sb[idx][:, ns_] if idx < HG else k_sb[:, ns_]
                ps_rot = (pool or psR).tile(
                    [128, 512], F32, tag=tag, name=f"rot{qq}_{idx}"
                )
                nc.tensor.matmul(
                    ps_rot[:, :], permRT_sb[:, :], tgt, start=True, stop=True
                )
                tmp = apool.tile([128, 512], BF16, tag="ropetmp", name=f"rt{qq}_{idx}")
                nc.vector.tensor_tensor(tmp[:, :], tgt, cosT[:, ns_], op=ALU.mult)
                nc.vector.tensor_tensor(tgt, ps_rot[:, :], sinT[:, ns_], op=ALU.mult)
                nc.vector.tensor_tensor(tgt, tgt, tmp[:, :], op=ALU.add)

            def emit_vt(stile):
                ps_v = psR.tile([128, 128], BF16, tag="vt", name=f"vt{stile}")
                nc.tensor.transpose(
                    ps_v[:, :],
                    vT_sb[:, stile * 128 : (stile + 1) * 128],
                    ident_bf[:, :],
                )
                nc.vector.tensor_copy(v_sb[stile][:, :], ps_v[:, :])

            def drain_one():
                if pending_items:
                    pending_items.pop(0)()

            for q in range(NQC):
                ns = slice(q * 512, (q + 1) * 512)
                if q > 0:
                    x_sb[q] = xspool.tile([128, NKC, 512], BF16, tag="x", name=f"x_{q}")
                    nc.sync.dma_start(
                        out=x_sb[q][:, :, :], in_=xh[:, :, ns]
                    )
                xq = x_sb[q]
                # sub-wave A: q heads 0..2 (3 PSUM banks); drain overlaps B
                psa = [
                    psA.tile([128, 512], F32, tag=f"pa{i}", name=f"pa{i}_{q}")
                    for i in range(3)
                ]
                for kt in range(NKC):
                    st, sp = kt == 0, kt == NKC - 1
                    for h in range(3):
                        nc.tensor.matmul(
                            psa[h][:, :],
                            wq_sb[:, kt, h * HD : (h + 1) * HD],
                            xq[:, kt, :],
                            start=st,
                            stop=sp,
                        )
                    if kt % 2 == 0:
                        drain_one()
                for h in range(3):
                    eng = nc.scalar if h % 2 == 0 else nc.vector
                    if eng is nc.scalar:
                        eng.activation(q_sb[h][:, ns], psa[h][:, :], AF.Copy)
                    else:
                        eng.tensor_copy(q_sb[h][:, ns], psa[h][:, :])
                # sub-wave B: q head 3, k, v (3 other banks)
                psb = [
                    psB.tile([128, 512], F32, tag=f"pb{i}", name=f"pb{i}_{q}")
                    for i in range(3)
                ]
                for kt in range(NKC):
                    st, sp = kt == 0, kt == NKC - 1
                    nc.tensor.matmul(
                        psb[0][:, :],
                        wq_sb[:, kt, 3 * HD : 4 * HD],
                        xq[:, kt, :],
                        start=st,
                        stop=sp,
                    )
                    nc.tensor.matmul(
                        psb[1][:, :], wk_sb[:, kt, :], xq[:, kt, :],
                        start=st, stop=sp,
                    )
                    nc.tensor.matmul(
                        psb[2][:, :], wv_sb[:, kt, :], xq[:, kt, :],
                        start=st, stop=sp,
                    )
                    if kt % 2 == 0:
                        drain_one()
                # vT first so deferred/immediate v-transposes unblock early
                nc.vector.tensor_copy(vT_sb[:, ns], psb[2][:, :])
                nc.scalar.activation(q_sb[3][:, ns], psb[0][:, :], AF.Copy)
                nc.vector.tensor_copy(k_sb[:, ns], psb[1][:, :])

                if q == NQC - 1:
                    # last chunk: v-transposes inline (psR dies with phase 1);
                    # rope drains into early attention via the psO "po" ring
                    for j in range(4):
                        emit_vt(q * 4 + j)
                else:
                    for j in range(4):
                        pending_items.append(
                            (lambda ss=q * 4 + j: emit_vt(ss))
                        )
                for idx in range(HG + 1):
                    if q == NQC - 1:
                        pending_items.append(
                            (lambda qq=q, ii=idx: emit_rope(
                                qq, ii, pool=psO_box[0], tag="po"
                            ))
                        )
                    else:
                        pending_items.append(
                            (lambda qq=q, ii=idx: emit_rope(qq, ii))
                        )

            psR.release()
            psB.release()
            psA.release()
            xspool.release()

            # o_proj weights: plain bf16 load, no dependencies — queue it
            # behind the x/w loads so it is resident long before the waves
            nc.sync.dma_start(out=wo_sb[:, :, :], in_=woh[:, :, :])

            # ---- phase 2: attention; AG(h) issued per head; o_proj waves ----
            ppool = tc.alloc_tile_pool(name="probs", bufs=6)
            pfpool = tc.alloc_tile_pool(name="pfold", bufs=3)
            ctxpool = tc.alloc_tile_pool(name="ctx", bufs=2)
            accpool = tc.alloc_tile_pool(name="acc", bufs=1)
            cblkpool = tc.alloc_tile_pool(name="cblk", bufs=1)
            ps2 = tc.alloc_tile_pool(name="ps2", bufs=1, space="PSUM")
            psO = tc.alloc_tile_pool(name="psO", bufs=2, space="PSUM")
            psO_box[0] = psO

            scale = float(HD**-0.5)

            # global software pipeline across the whole attention sweep:
            # each kt "slot" emits scores+exp; queued sums/ctx (lag 2) and
            # chunk-finalize work (DVE lag 0 / PE-norm lag 4) retire later so
            # the PE never waits head-of-line on exp or the DVE norm chain.
            slot_box = [0]
            pend = []  # entries: (slot, lag, is_pe, fn); fn emits instructions

            def pump(force=False):
                ran_pe = False
                while pend:
                    s0, lag, is_pe, fn = pend[0]
                    if not is_pe:
                        pend.pop(0)
                        fn()
                        continue
                    if ran_pe and not force:
                        break
                    if force or slot_box[0] - s0 >= lag:
                        pend.pop(0)
                        fn()
                        ran_pe = True
                        continue
                    break

            def emit_norm(hh, qq, craw, rc):
                ps_rb = ps2.tile(
                    [128, 512], F32, tag="scores", name=f"rb{hh}_{qq}", bufs=3
                )
                nc.tensor.matmul(
                    ps_rb[:, :], ones_row_bf[:, :], rc[:, :],
                    start=True, stop=True,
                )
                csb = ctxpool.tile(
                    [128, 512], BF16, tag="ctxsb", name=f"cs{hh}_{qq}"
                )
                nc.vector.tensor_tensor(
                    csb[:, :], craw[:, :], ps_rb[:, :], op=ALU.mult
                )
                # collective input stores ride the GpSimd queue (where the
                # AG triggers already live) so they are never queued behind
                # bulk weight/x loads on the sync DMA queue.
                if hh == HG - 1 and qq >= 2:
                    jq = qq - 2
                    nc.gpsimd.dma_start(out=cc_in_q[jq][:, :], in_=csb[:, :])
                    nc.gpsimd.collective_compute(
                        "AllGather",
                        mybir.AluOpType.bypass,
                        replica_groups=GROUPS,
                        ins=[cc_in_q[jq][:, :]],
                        outs=[cc_out_q[jq][:, :]],
                    )
                    return
                hhf = qq // 2
                nc.gpsimd.dma_start(
                    out=cc_in[hh][hhf][:, (qq % 2) * 512 : (qq % 2 + 1) * 512],
                    in_=csb[:, :],
                )
                if qq % 2 == 1:
                    nc.gpsimd.collective_compute(
                        "AllGather",
                        mybir.AluOpType.bypass,
                        replica_groups=GROUPS,
                        ins=[cc_in[hh][hhf][:, :]],
                        outs=[cc_out[hh][hhf][:, :]],
                    )

            for h in range(HG):
                for q in range(NQC):
                    nkt = 4 * q + 4
                    ps_sums = ps2.tile(
                        [1, 512], F32, tag="sums", name=f"sums{h}_{q}", bufs=1
                    )
                    ps_ctx = ps2.tile(
                        [128, 512], F32, tag="ctx", name=f"ctx{h}_{q}", bufs=2
                    )
                    probs = {}
                    pairs = {}

                    def make_sums_ctx(hh, qq, kt_, psums, pctx, probs_t, sums_st):
                        def fn():
                            c0_ = max(0, kt_ - 4 * qq) * 128
                            cs_ = slice(c0_, 512)
                            nkt_ = 4 * qq + 4
                            sp_ = kt_ == nkt_ - 1
                            nc.tensor.matmul(
                                psums[:, cs_], ones_col_sb[:, :], probs_t[:, cs_],
                                start=sums_st, stop=sp_,
                            )
                            nc.tensor.matmul(
                                pctx[:, cs_], v_sb[kt_][:, :], probs_t[:, cs_],
                                start=kt_ == 0, stop=sp_,
                            )
                        return fn

                    def make_ctx_pair(hh, qq, ka, kb, pa, pb, pctx):
                        def fn():
                            nc.tensor.matmul(
                                pctx[:, :], v_sb[ka][:, :], pa[:, :],
                                start=ka == 0, stop=False,
                            )
                            nc.tensor.matmul(
                                pctx[:, :], v_sb[kb][:, :], pb[:, :],
                                start=False, stop=False,
                            )
                        return fn

                    def make_quad_sums(hh, qq, pq, psums, sums_st):
                        def fn():
                            nc.tensor.matmul(
                                psums[:, :], ones_col_sb[:, :], pq[:, :],
                                start=sums_st, stop=False,
                            )
                        return fn

                    for kt in range(nkt):
                        o = kt - 4 * q
                        c0 = max(0, o) * 128  # first valid column in the chunk
                        cs = slice(c0, 512)
                        ps_s = ps2.tile(
                            [128, 512], F32, tag="scores", name=f"s{h}_{q}_{kt}", bufs=3
                        )
                        nc.tensor.matmul(
                            ps_s[:, cs],
                            k_sb[:, kt * 128 : (kt + 1) * 128],
                            q_sb[h][:, q * 512 + c0 : (q + 1) * 512],
                            start=True,
                            stop=True,
                        )
                        pT = ppool.tile(
                            [128, 512], BF16, tag="probs", name=f"p{h}_{q}_{kt}"
                        )
                        nc.scalar.activation(pT[:, cs], ps_s[:, cs], AF.Exp, scale=scale)
                        if o >= 0:
                            # causal mask: zero probs where key > query within
                            # the 128-col diagonal subtile (cheap DVE multiply
                            # instead of a PE bias matmul)
                            nc.vector.tensor_tensor(
                                pT[:, c0 : c0 + 128], pT[:, c0 : c0 + 128],
                                mtri_sb[:, :], op=ALU.mult,
                            )
                        probs[kt] = pT
                        drain_one()
                        if o < 0 and kt % 2 == 1:
                            # fold the completed pair on DVE; pairs feed ctx
                            # matmuls directly and quad-folded sums matmuls
                            pf = pfpool.tile(
                                [128, 512], BF16, tag="pfold",
                                name=f"pf{h}_{q}_{kt}", bufs=3,
                            )
                            nc.vector.tensor_tensor(
                                pf[:, :], probs[kt - 1][:, :], probs[kt][:, :],
                                op=ALU.add,
                            )
                            pairs[kt // 2] = pf
                            pend.append(
                                (slot_box[0], 2, True,
                                 make_ctx_pair(h, q, kt - 1, kt, probs.pop(kt - 1),
                                               probs.pop(kt), ps_ctx))
                            )
                            if kt % 4 == 3:
                                # quad fold: one sums matmul covers 4 kts
                                pq = pfpool.tile(
                                    [128, 512], BF16, tag="qfold",
                                    name=f"pq{h}_{q}_{kt}", bufs=2,
                                )
                                nc.vector.tensor_tensor(
                                    pq[:, :], pairs.pop(kt // 2 - 1)[:, :],
                                    pairs.pop(kt // 2)[:, :], op=ALU.add,
                                )
                                pend.append(
                                    (slot_box[0], 2, True,
                                     make_quad_sums(h, q, pq, ps_sums, kt == 3))
                                )
                        elif o >= 0:
                            pend.append(
                                (slot_box[0], 2, True,
                                 make_sums_ctx(h, q, kt, ps_sums, ps_ctx,
                                               probs.pop(kt),
                                               q == 0 and kt == 0))
                            )
                        slot_box[0] += 1
                        pump()

                    def make_fin_dve(hh, qq, psums, pctx):
                        box = {}

                        def fn():
                            craw = apool.tile(
                                [128, 512], F32, tag="ctx_raw", name=f"cr{hh}_{qq}"
                            )
                            nc.vector.tensor_copy(craw[:, :], pctx[:, :])
                            rf = apool.tile(
                                [1, 512], F32, tag="recipf", name=f"rf{hh}_{qq}"
                            )
                            nc.vector.reciprocal_approx_fast(rf[:, :], psums[:, :])
                            rc = apool.tile(
                                [1, 512], BF16, tag="recip", name=f"rc{hh}_{qq}"
                            )
                            nc.vector.tensor_copy(rc[:, :], rf[:, :])
                            box["craw"] = craw
                            box["rc"] = rc
                        return box, fn

                    box, fin_dve = make_fin_dve(h, q, ps_sums, ps_ctx)
                    if h == HG - 1 and q >= 2:
                        # tail chunks: flush the pipeline and emit the norm +
                        # collective input eagerly so the last AllGathers fire
                        # as soon as the data exists (they gate the final
                        # o_proj waves)
                        pump(force=True)
                        fin_dve()
                        emit_norm(h, q, box["craw"], box["rc"])
                        continue
                    pend.append((slot_box[0], 0, False, fin_dve))

                    def make_fin_pe(hh, qq, bx):
                        def fn():
                            emit_norm(hh, qq, bx["craw"], bx["rc"])
                        return fn

                    pend.append((slot_box[0], 4, True, make_fin_pe(h, q, box)))
            pump(force=True)

            # o_proj waves: wave h brings rows for global kt = 4r + h
            acc_sb = [
                accpool.tile([128, OC], F32, tag=f"acc{i}", name=f"acc{i}")
                for i in range(NST)
            ]

            def emit_half_wave(h, hf):
                cblk = []
                for r in range(TP):
                    t = cblkpool.tile(
                        [128, S // 2], BF16, tag=f"cblk{r}",
                        name=f"cb{h}_{hf}_{r}", bufs=2,
                    )
                    nc.sync.dma_start(
                        out=t[:, :], in_=cc_out[h][hf][r * 128 : (r + 1) * 128, :]
                    )
                    cblk.append(t)
                for j in range(NST // 2):
                    stile = hf * (NST // 2) + j
                    ps_po = psO.tile(
                        [128, OC], F32, tag="po", name=f"po{h}_{hf}_{j}"
                    )
                    for r in range(TP):
                        nc.tensor.matmul(
                            ps_po[:, :],
                            cblk[r][:, j * 128 : (j + 1) * 128],
                            wo_sb[:, 4 * r + h, :],
                            start=r == 0,
                            stop=r == TP - 1,
                        )
                    if h == 0:
                        nc.vector.tensor_copy(acc_sb[stile][:, :], ps_po[:, :])
                    else:
                        nc.vector.tensor_tensor(
                            acc_sb[stile][:, :], acc_sb[stile][:, :], ps_po[:, :],
                            op=ALU.add,
                        )
                    if h == HG - 1:
                        nc.sync.dma_start(
                            out=out[stile * 128 : (stile + 1) * 128, :],
                            in_=acc_sb[stile][:, :],
                        )

            for h in range(HG):
                for hf in range(2):
                    if h == HG - 1 and hf == 1:
                        continue  # final s-tiles arrive as quarter gathers
                    emit_half_wave(h, hf)
            # final quarter waves: head HG-1, s-tiles 8..15
            h = HG - 1
            for jq in (0, 1):
                cblk = []
                for r in range(TP):
                    t = cblkpool.tile(
                        [128, 512], BF16, tag=f"cblk{r}", name=f"cq{jq}_{r}", bufs=2
                    )
                    nc.sync.dma_start(
                        out=t[:, :], in_=cc_out_q[jq][r * 128 : (r + 1) * 128, :]
                    )
                    cblk.append(t)
                for j in range(4):
                    stile = 8 + jq * 4 + j
                    ps_po = psO.tile([128, OC], F32, tag="po", name=f"poq{jq}_{j}")
                    for r in range(TP):
                        nc.tensor.matmul(
                            ps_po[:, :],
                            cblk[r][:, j * 128 : (j + 1) * 128],
                            wo_sb[:, 4 * r + h, :],
                            start=r == 0,
                            stop=r == TP - 1,
                        )
                    nc.vector.tensor_tensor(
                        acc_sb[stile][:, :], acc_sb[stile][:, :], ps_po[:, :],
                        op=ALU.add,
                    )
                    nc.sync.dma_start(
                        out=out[stile * 128 : (stile + 1) * 128, :],
                        in_=acc_sb[stile][:, :],
                    )
            psO.release()
            ps2.release()
            cblkpool.release()
            accpool.release()
            ctxpool.release()
            pfpool.release()
            ppool.release()
            qkvpool.release()

    nc.compile()
    return nc


def _get_nc():
    if "nc" not in _CACHE:
        _CACHE["nc"] = _build()
    return _CACHE["nc"]


def _shard(hidden_states, position_ids, Wq, Wkv, Wo):
    """Host-side layout + bf16 conversion per core. No device work."""
    bf = ml_dtypes.bfloat16
    x = np.asarray(hidden_states, np.float32)
    pos = np.asarray(position_ids, np.int32)
    Wq = np.asarray(Wq, np.float32)
    Wkv = np.asarray(Wkv, np.float32)
    Wo = np.asarray(Wo, np.float32)

    def pkt(m):  # [free, HID] f32 -> [128, NKC, free] bf16
        # m.T is [HID, free]; row r = kt*128 + p
        t = np.ascontiguousarray(m.T).astype(bf)
        return np.ascontiguousarray(
            t.reshape(NKC, 128, m.shape[0]).transpose(1, 0, 2)
        )

    in_maps = []
    xs = [pkt(x[b]) for b in range(B)]
    wq_g = [pkt(Wq[g * OC : (g + 1) * OC]) for g in range(TP)]
    wo_g = [pkt(Wo[g * OC : (g + 1) * OC]) for g in range(TP)]
    wk_g = []
    wv_g = []
    for g in range(TP):
        krows = g * 2 * HD + 2 * np.arange(HD)
        wk_g.append(pkt(Wkv[krows]))
        wv_g.append(pkt(Wkv[krows + 1]))
    # rope tables per batch: [HD, S] with the half-dim frequencies tiled x2
    half = HD // 2
    invf = (1.0 / (ROPE_BASE ** (np.arange(half) / half))).astype(np.float32)
    invf2 = np.tile(invf, 2)[:, None]  # [HD, 1]
    sin_b, cos_b = [], []
    for b in range(B):
        ang = invf2 * pos[b][None, :].astype(np.float32)  # [HD, S]
        sin_b.append(np.ascontiguousarray(np.sin(ang)).astype(bf))
        cos_b.append(np.ascontiguousarray(np.cos(ang)).astype(bf))
    for c in range(N_CORES):
        b, g = c // TP, c % TP
        in_maps.append(
            {
                "xh": xs[b],
                "wqh": wq_g[g],
                "wkh": wk_g[g],
                "wvh": wv_g[g],
                "woh": wo_g[g],
                "sinh": sin_b[b],
                "cosh": cos_b[b],
            }
        )
    return in_maps


def run(hidden_states, position_ids, Wq, Wkv, Wo, trace=False):
    nc = _get_nc()
    in_maps = _shard(hidden_states, position_ids, Wq, Wkv, Wo)
    res = bass_utils.run_bass_kernel_spmd(
        nc, in_maps, core_ids=list(range(N_CORES)), trace=trace
    )
    out = np.empty((B, S, HID), np.float32)
    for c in range(N_CORES):
        b, g = c // TP, c % TP
        out[b][:, g * OC : (g + 1) * OC] = res.results[c]["out_slice"]
    return out, res


def kernel(hidden_states, position_ids, Wq, Wkv, Wo):
    out, _ = run(hidden_states, position_ids, Wq, Wkv, Wo, trace=False)
    return out


# revision 21
# speedup vs baseline: 1.0634x; 1.0634x over previous
"""Trainium2 Bass kernel for nn_MultiHeadAttention_3539053052118.

GQA attention (B=2, S=2048, HID=2048, 16 q-heads, 4 kv-heads, RoPE, causal)
distributed over 8 NeuronCores: 2-way data-parallel over batch x 4-way
tensor-parallel over kv-head groups. Each core computes q/kv projections for
its 4 q-heads + 1 kv-head (bf16 matmuls; inputs are pre-converted to bf16 and
pre-laid-out on the host so no on-chip casts are needed), RoPE, causal flash
attention with a globally software-pipelined scores->exp->sums/ctx chain;
each head's context is AllGather-ed (bf16) within the 4-core batch group as
soon as it is ready (collective input DMAs ride the otherwise-idle GpSimd
queue so they are never stuck behind bulk loads), and the o_proj accumulates
per-wave into SBUF so the collectives overlap attention. Each core produces a
distinct 512-column slice of the output.
"""

import math
import sys
import types

sys.path.insert(0, "/opt/trn_rl_repo")

import antenv  # noqa: F401

if "antenv.axon_hooks" not in sys.modules:
    _hooks = types.ModuleType("antenv.axon_hooks")
    _hook_box = {"hook": None}
    _hooks.set_axon_ntff_profile_hook = lambda h: _hook_box.__setitem__("hook", h)
    _hooks.get_axon_ntff_profile_hook = lambda: _hook_box["hook"]
    sys.modules["antenv.axon_hooks"] = _hooks
    try:
        from trn_agent_boot.trn_boot import _ntff_profile_via_ctypes

        _hooks.set_axon_ntff_profile_hook(
            _ntff_profile_via_ctypes("/opt/axon/libaxon_pjrt.so")
        )
    except Exception:
        pass

import numpy as np
import ml_dtypes
import concourse.bass as bass
import concourse.mybir as mybir
import concourse.tile as tile
from concourse import bacc
from concourse import bass_utils
from concourse.masks import make_identity

F32 = mybir.dt.float32
F32R = mybir.dt.float32r
BF16 = mybir.dt.bfloat16
I32 = mybir.dt.int32
AF = mybir.ActivationFunctionType
ALU = mybir.AluOpType

B, S, HID = 2, 2048, 2048
NH, NKV = 16, 4
HD = 128
ROPE_BASE = 10000.0
PI = math.pi

N_CORES = 8
TP = 4
HG = NH // TP  # 4 q heads per core
GROUPS = [[0, 1, 2, 3], [4, 5, 6, 7]]

NKC = HID // 128  # 16 contraction tiles
NQC = S // 512  # 4 q/n chunks
NST = S // 128  # 16 s tiles
OC = 512  # output columns per core

_CACHE = {}


def _build():
    nc = bacc.Bacc("TRN2", target_bir_lowering=False, debug=False, num_devices=N_CORES)

    # host-prepared bf16 inputs, already in [partition, ktile, free] layout
    xh = nc.dram_tensor("xh", [128, NKC, S], BF16, kind="ExternalInput").ap()
    wqh = nc.dram_tensor("wqh", [128, NKC, HG * HD], BF16, kind="ExternalInput").ap()
    wkh = nc.dram_tensor("wkh", [128, NKC, HD], BF16, kind="ExternalInput").ap()
    wvh = nc.dram_tensor("wvh", [128, NKC, HD], BF16, kind="ExternalInput").ap()
    woh = nc.dram_tensor("woh", [128, NKC, OC], BF16, kind="ExternalInput").ap()
    # rope tables, computed on host from position_ids (valid for arbitrary
    # positions; avoids the on-device sin-range reduction chain entirely)
    sinh = nc.dram_tensor("sinh", [HD, S], BF16, kind="ExternalInput").ap()
    cosh = nc.dram_tensor("cosh", [HD, S], BF16, kind="ExternalInput").ap()
    out = nc.dram_tensor("out_slice", [S, OC], F32, kind="ExternalOutput").ap()

    # per-head collective bounce buffers (separate tensors so AG(h) only
    # depends on head h's writes)
    cc_in = [
        [nc.dram_tensor(f"cc_in{h}_{hf}", [HD, S // 2], BF16).ap() for hf in range(2)]
        for h in range(HG)
    ]
    cc_out = [
        [
            nc.dram_tensor(f"cc_out{h}_{hf}", [TP * HD, S // 2], BF16).ap()
            for hf in range(2)
        ]
        for h in range(HG)
    ]
    # quarter-granularity buffers for the very last gathers (tail latency)
    cc_in_q = [nc.dram_tensor(f"cc_inq{j}", [HD, 512], BF16).ap() for j in range(2)]
    cc_out_q = [
        nc.dram_tensor(f"cc_outq{j}", [TP * HD, 512], BF16).ap() for j in range(2)
    ]
    # tiny warm-up collective: absorbs first-collective setup cost and
    # re-syncs the cores right at kernel start
    cc_wout = nc.dram_tensor("cc_wout", [TP, 64], BF16).ap()

    # ---- inline constants ----
    half = HD // 2
    R = np.zeros((HD, HD), np.float32)
    for p in range(half):
        R[p, p + half] = -1.0
    for p in range(half, HD):
        R[p, p - half] = 1.0
    permRT_c = nc.inline_tensor(
        np.ascontiguousarray(R.T).astype(ml_dtypes.bfloat16), "permRT"
    ).ap()
    ones_row_bf_c = nc.inline_tensor(
        np.ones((1, 128), ml_dtypes.bfloat16), "ones_row_bf"
    ).ap()
    ones_col_c = nc.inline_tensor(
        np.ones((128, 1), ml_dtypes.bfloat16), "ones_col"
    ).ap()
    # causal mask: M[p, j] = 0 where key p > query j (within diag subtile)
    mtri = np.where(
        np.arange(128)[:, None] > np.arange(128)[None, :], 0.0, 1.0
    ).astype(ml_dtypes.bfloat16)
    mtri_c = nc.inline_tensor(mtri, "mtri").ap()
    ident_bf_c = nc.inline_tensor(
        np.eye(128, dtype=ml_dtypes.bfloat16), "ident_bf"
    ).ap()
    warm_c = nc.inline_tensor(np.ones((1, 64), ml_dtypes.bfloat16), "warm").ap()

    with tile.TileContext(nc) as tc:
        with (
            tc.tile_pool(name="const", bufs=1) as cpool,
            tc.tile_pool(name="w", bufs=1) as wpool,
            tc.tile_pool(name="attn", bufs=2) as apool,
        ):
            qkvpool = tc.alloc_tile_pool(name="qkv", bufs=1)
            # ---- constants ----
            permRT_sb = cpool.tile([HD, HD], BF16)
            nc.scalar.dma_start(out=permRT_sb[:, :], in_=permRT_c[:, :])
            ones_row_bf = cpool.tile([1, 128], BF16)
            nc.scalar.dma_start(out=ones_row_bf[:, :], in_=ones_row_bf_c[:, :])
            ones_col_sb = cpool.tile([128, 1], BF16)
            nc.scalar.dma_start(out=ones_col_sb[:, :], in_=ones_col_c[:, :])
            mtri_sb = cpool.tile([128, 128], BF16)
            nc.scalar.dma_start(out=mtri_sb[:, :], in_=mtri_c[:, :])
            ident_bf = cpool.tile([128, 128], BF16)
            nc.scalar.dma_start(out=ident_bf[:, :], in_=ident_bf_c[:, :])

            # warm-up AllGather: first in the CC queue, runs during phase 0/1
            nc.gpsimd.collective_compute(
                "AllGather",
                mybir.AluOpType.bypass,
                replica_groups=GROUPS,
                ins=[warm_c[:, :]],
                outs=[cc_wout[:, :]],
            )

            # host-computed rope tables (loaded after the chunk-0 operands,
            # see below — they are not needed until the first rope)
            sinT = cpool.tile([128, S], BF16, tag="tab_sin", name="tab_sin")
            cosT = cpool.tile([128, S], BF16, tag="tab_cos", name="tab_cos")

            # ---- persistent weights (bf16, direct DMA, no casts) ----
            wq_sb = wpool.tile([128, NKC, HG * HD], BF16, tag="wq", name="wq_sb")
            wk_sb = wpool.tile([128, NKC, HD], BF16, tag="wk", name="wk_sb")
            wv_sb = wpool.tile([128, NKC, HD], BF16, tag="wv", name="wv_sb")
            wo_sb = wpool.tile([128, NKC, OC], BF16, tag="wo", name="wo_sb")
            # persistent qkv storage (bf16)
            q_sb = [
                qkvpool.tile([128, S], BF16, tag=f"q{h}", name=f"q{h}")
                for h in range(HG)
            ]
            k_sb = qkvpool.tile([128, S], BF16, tag="k", name="k_sb")
            vT_sb = qkvpool.tile([128, S], BF16, tag="vT", name="vT_sb")
            v_sb = [
                qkvpool.tile([128, HD], BF16, tag=f"v{i}", name=f"v{i}")
                for i in range(NST)
            ]

            xspool = tc.alloc_tile_pool(name="xs", bufs=2)
            psA = tc.alloc_tile_pool(name="psA", bufs=1, space="PSUM")
            psB = tc.alloc_tile_pool(name="psB", bufs=1, space="PSUM")
            psR = tc.alloc_tile_pool(name="psR", bufs=1, space="PSUM")

            # interleave chunk-0 x with weights so the kt=0 operands land
            # first; everything is already bf16 so DMAs feed matmuls directly.
            # wk/wv are only needed by sub-wave B (after all 16 kts of A) and
            # the rope tables only at the first rope, so they load after.
            x_sb = [None] * NQC
            x_sb[0] = xspool.tile([128, NKC, 512], BF16, tag="x", name="x_0")
            for j in range(4):
                ks = slice(j * 4, (j + 1) * 4)
                nc.sync.dma_start(out=x_sb[0][:, ks, :], in_=xh[:, ks, 0:512])
                nc.sync.dma_start(out=wq_sb[:, ks, :], in_=wqh[:, ks, :])
            nc.sync.dma_start(out=wk_sb[:, :, :], in_=wkh[:, :, :])
            nc.sync.dma_start(out=wv_sb[:, :, :], in_=wvh[:, :, :])

            # ---- phase 1: projections + rope + v transpose ----
            # rope/v-transpose of chunk q is deferred and interleaved into the
            # PE stream of chunk q+1 (or early attention) so the PE never
            # waits head-of-line on the DVE rope chain.
            pending_items = []  # closures emitting one deferred PE item each
            psO_box = [None]  # filled once the attention-phase psO pool exists

            def emit_rope(qq, idx, pool=None, tag="rot"):
                ns_ = slice(qq * 512, (qq + 1) * 512)
                tgt = q_sb[idx][:, ns_] if idx < HG else k_sb[:, ns_]
                ps_rot = (pool or psR).tile(
                    [128, 512], F32, tag=tag, name=f"rot{qq}_{idx}"
                )
                nc.tensor.matmul(
                    ps_rot[:, :], permRT_sb[:, :], tgt, start=True, stop=True
                )
                tmp = apool.tile([128, 512], BF16, tag="ropetmp", name=f"rt{qq}_{idx}")
                nc.vector.tensor_tensor(tmp[:, :], tgt, cosT[:, ns_], op=ALU.mult)
                nc.vector.tensor_tensor(tgt, ps_rot[:, :], sinT[:, ns_], op=ALU.mult)
                nc.vector.tensor_tensor(tgt, tgt, tmp[:, :], op=ALU.add)

            def emit_vt(stile):
                ps_v = psR.tile([128, 128], BF16, tag="vt", name=f"vt{stile}")
                nc.tensor.transpose(
                    ps_v[:, :],
                    vT_sb[:, stile * 128 : (stile + 1) * 128],
                    ident_bf[:, :],
                )
                nc.vector.tensor_copy(v_sb[stile][:, :], ps_v[:, :])

            def drain_one():
                if pending_items:
                    pending_items.pop(0)()

            for q in range(NQC):
                ns = slice(q * 512, (q + 1) * 512)
                if q > 0:
                    x_sb[q] = xspool.tile([128, NKC, 512], BF16, tag="x", name=f"x_{q}")
                    nc.sync.dma_start(
                        out=x_sb[q][:, :, :], in_=xh[:, :, ns]
                    )
                    if q == 1:
                        # rope tables: needed first by rope(0) deferred into
                        # this chunk; load behind x_1 so the chunk-0/1
                        # operands keep DMA priority
                        nc.sync.dma_start(out=sinT[:, :], in_=sinh[:, :])
                        nc.sync.dma_start(out=cosT[:, :], in_=cosh[:, :])
                xq = x_sb[q]
                # sub-wave A: q heads 0..2 (3 PSUM banks); drain overlaps B
                psa = [
                    psA.tile([128, 512], F32, tag=f"pa{i}", name=f"pa{i}_{q}")
                    for i in range(3)
                ]
                for kt in range(NKC):
                    st, sp = kt == 0, kt == NKC - 1
                    for h in range(3):
                        nc.tensor.matmul(
                            psa[h][:, :],
                            wq_sb[:, kt, h * HD : (h + 1) * HD],
                            xq[:, kt, :],
                            start=st,
                            stop=sp,
                        )
                    if kt % 2 == 0:
                        drain_one()
                for h in range(3):
                    eng = nc.scalar if h % 2 == 0 else nc.vector
                    if eng is nc.scalar:
                        eng.activation(q_sb[h][:, ns], psa[h][:, :], AF.Copy)
                    else:
                        eng.tensor_copy(q_sb[h][:, ns], psa[h][:, :])
                # sub-wave B: q head 3, k, v (3 other banks)
                psb = [
                    psB.tile([128, 512], F32, tag=f"pb{i}", name=f"pb{i}_{q}")
                    for i in range(3)
                ]
                for kt in range(NKC):
                    st, sp = kt == 0, kt == NKC - 1
                    nc.tensor.matmul(
                        psb[0][:, :],
                        wq_sb[:, kt, 3 * HD : 4 * HD],
                        xq[:, kt, :],
                        start=st,
                        stop=sp,
                    )
                    nc.tensor.matmul(
                        psb[1][:, :], wk_sb[:, kt, :], xq[:, kt, :],
                        start=st, stop=sp,
                    )
                    nc.tensor.matmul(
                        psb[2][:, :], wv_sb[:, kt, :], xq[:, kt, :],
                        start=st, stop=sp,
                    )
                    if kt % 2 == 0:
                        drain_one()
                # vT first so deferred/immediate v-transposes unblock early
                nc.vector.tensor_copy(vT_sb[:, ns], psb[2][:, :])
                nc.scalar.activation(q_sb[3][:, ns], psb[0][:, :], AF.Copy)
                nc.vector.tensor_copy(k_sb[:, ns], psb[1][:, :])

                if q == NQC - 1:
                    # last chunk: v-transposes inline (psR dies with phase 1);
                    # rope drains into early attention via the psO "po" ring
                    for j in range(4):
                        emit_vt(q * 4 + j)
                else:
                    for j in range(4):
                        pending_items.append(
                            (lambda ss=q * 4 + j: emit_vt(ss))
                        )
                for idx in range(HG + 1):
                    if q == NQC - 1:
                        pending_items.append(
                            (lambda qq=q, ii=idx: emit_rope(
                                qq, ii, pool=psO_box[0], tag="po"
                            ))
                        )
                    else:
                        pending_items.append(
                            (lambda qq=q, ii=idx: emit_rope(qq, ii))
                        )

            psR.release()
            psB.release()
            psA.release()
            xspool.release()

            # o_proj weights: plain bf16 load, no dependencies — queue it
            # behind the x/w loads so it is resident long before the waves
            nc.sync.dma_start(out=wo_sb[:, :, :], in_=woh[:, :, :])

            # ---- phase 2: attention; AG(h) issued per head; o_proj waves ----
            ppool = tc.alloc_tile_pool(name="probs", bufs=6)
            pfpool = tc.alloc_tile_pool(name="pfold", bufs=3)
            ctxpool = tc.alloc_tile_pool(name="ctx", bufs=2)
            accpool = tc.alloc_tile_pool(name="acc", bufs=1)
            cblkpool = tc.alloc_tile_pool(name="cblk", bufs=1)
            ps2 = tc.alloc_tile_pool(name="ps2", bufs=1, space="PSUM")
            psO = tc.alloc_tile_pool(name="psO", bufs=2, space="PSUM")
            psO_box[0] = psO

            scale = float(HD**-0.5)

            # global software pipeline across the whole attention sweep:
            # each kt "slot" emits scores+exp; queued sums/ctx (lag 2) and
            # chunk-finalize work (DVE lag 0 / PE-norm lag 4) retire later so
            # the PE never waits head-of-line on exp or the DVE norm chain.
            slot_box = [0]
            pend = []  # entries: (slot, lag, is_pe, fn); fn emits instructions

            def pump(force=False):
                ran_pe = False
                while pend:
                    s0, lag, is_pe, fn = pend[0]
                    if not is_pe:
                        pend.pop(0)
                        fn()
                        continue
                    if ran_pe and not force:
                        break
                    if force or slot_box[0] - s0 >= lag:
                        pend.pop(0)
                        fn()
                        ran_pe = True
                        continue
                    break

            def emit_norm(hh, qq, craw, rc):
                ps_rb = ps2.tile(
                    [128, 512], F32, tag="scores", name=f"rb{hh}_{qq}", bufs=3
                )
                nc.tensor.matmul(
                    ps_rb[:, :], ones_row_bf[:, :], rc[:, :],
                    start=True, stop=True,
                )
                csb = ctxpool.tile(
                    [128, 512], BF16, tag="ctxsb", name=f"cs{hh}_{qq}"
                )
                nc.vector.tensor_tensor(
                    csb[:, :], craw[:, :], ps_rb[:, :], op=ALU.mult
                )
                # collective input stores ride the sync queue: at this point
                # it carries nothing blocking (wo is dependency-free and
                # loaded early; cblk loads all come later in program order),
                # and keeping them off the GpSimd queue means the compute
                # pipeline never backs up behind collective completion.
                if hh == HG - 1 and qq >= 2:
                    jq = qq - 2
                    nc.sync.dma_start(out=cc_in_q[jq][:, :], in_=csb[:, :])
                    nc.gpsimd.collective_compute(
                        "AllGather",
                        mybir.AluOpType.bypass,
                        replica_groups=GROUPS,
                        ins=[cc_in_q[jq][:, :]],
                        outs=[cc_out_q[jq][:, :]],
                    )
                    return
                hhf = qq // 2
                nc.sync.dma_start(
                    out=cc_in[hh][hhf][:, (qq % 2) * 512 : (qq % 2 + 1) * 512],
                    in_=csb[:, :],
                )
                if qq % 2 == 1:
                    nc.gpsimd.collective_compute(
                        "AllGather",
                        mybir.AluOpType.bypass,
                        replica_groups=GROUPS,
                        ins=[cc_in[hh][hhf][:, :]],
                        outs=[cc_out[hh][hhf][:, :]],
                    )

            for h in range(HG):
                for q in range(NQC):
                    nkt = 4 * q + 4
                    ps_sums = ps2.tile(
                        [1, 512], F32, tag="sums", name=f"sums{h}_{q}", bufs=1
                    )
                    ps_ctx = ps2.tile(
                        [128, 512], F32, tag="ctx", name=f"ctx{h}_{q}", bufs=2
                    )
                    probs = {}
                    pairs = {}

                    def make_sums_ctx(hh, qq, kt_, psums, pctx, probs_t, sums_st):
                        def fn():
                            c0_ = max(0, kt_ - 4 * qq) * 128
                            cs_ = slice(c0_, 512)
                            nkt_ = 4 * qq + 4
                            sp_ = kt_ == nkt_ - 1
                            nc.tensor.matmul(
                                psums[:, cs_], ones_col_sb[:, :], probs_t[:, cs_],
                                start=sums_st, stop=sp_,
                            )
                            nc.tensor.matmul(
                                pctx[:, cs_], v_sb[kt_][:, :], probs_t[:, cs_],
                                start=kt_ == 0, stop=sp_,
                            )
                        return fn

                    def make_ctx_pair(hh, qq, ka, kb, pa, pb, pctx):
                        def fn():
                            nc.tensor.matmul(
                                pctx[:, :], v_sb[ka][:, :], pa[:, :],
                                start=ka == 0, stop=False,
                            )
                            nc.tensor.matmul(
                                pctx[:, :], v_sb[kb][:, :], pb[:, :],
                                start=False, stop=False,
                            )
                        return fn

                    def make_quad_sums(hh, qq, pq, psums, sums_st):
                        def fn():
                            nc.tensor.matmul(
                                psums[:, :], ones_col_sb[:, :], pq[:, :],
                                start=sums_st, stop=False,
                            )
                        return fn

                    for kt in range(nkt):
                        o = kt - 4 * q
                        c0 = max(0, o) * 128  # first valid column in the chunk
                        cs = slice(c0, 512)
                        ps_s = ps2.tile(
                            [128, 512], F32, tag="scores", name=f"s{h}_{q}_{kt}", bufs=3
                        )
                        nc.tensor.matmul(
                            ps_s[:, cs],
                            k_sb[:, kt * 128 : (kt + 1) * 128],
                            q_sb[h][:, q * 512 + c0 : (q + 1) * 512],
                            start=True,
                            stop=True,
                        )
                        pT = ppool.tile(
                            [128, 512], BF16, tag="probs", name=f"p{h}_{q}_{kt}"
                        )
                        nc.scalar.activation(pT[:, cs], ps_s[:, cs], AF.Exp, scale=scale)
                        if o >= 0:
                            # causal mask: zero probs where key > query within
                            # the 128-col diagonal subtile (cheap DVE multiply
                            # instead of a PE bias matmul)
                            nc.vector.tensor_tensor(
                                pT[:, c0 : c0 + 128], pT[:, c0 : c0 + 128],
                                mtri_sb[:, :], op=ALU.mult,
                            )
                        probs[kt] = pT
                        drain_one()
                        if o < 0 and kt % 2 == 1:
                            # fold the completed pair on DVE; pairs feed ctx
                            # matmuls directly and quad-folded sums matmuls
                            pf = pfpool.tile(
                                [128, 512], BF16, tag="pfold",
                                name=f"pf{h}_{q}_{kt}", bufs=3,
                            )
                            nc.vector.tensor_tensor(
                                pf[:, :], probs[kt - 1][:, :], probs[kt][:, :],
                                op=ALU.add,
                            )
                            pairs[kt // 2] = pf
                            pend.append(
                                (slot_box[0], 2, True,
                                 make_ctx_pair(h, q, kt - 1, kt, probs.pop(kt - 1),
                                               probs.pop(kt), ps_ctx))
                            )
                            if kt % 4 == 3:
                                # quad fold: one sums matmul covers 4 kts
                                pq = pfpool.tile(
                                    [128, 512], BF16, tag="qfold",
                                    name=f"pq{h}_{q}_{kt}", bufs=2,
                                )
                                nc.vector.tensor_tensor(
                                    pq[:, :], pairs.pop(kt // 2 - 1)[:, :],
                                    pairs.pop(kt // 2)[:, :], op=ALU.add,
                                )
                                pend.append(
                                    (slot_box[0], 2, True,
                                     make_quad_sums(h, q, pq, ps_sums, kt == 3))
                                )
                        elif o >= 0:
                            pend.append(
                                (slot_box[0], 2, True,
                                 make_sums_ctx(h, q, kt, ps_sums, ps_ctx,
                                               probs.pop(kt),
                                               q == 0 and kt == 0))
                            )
                        slot_box[0] += 1
                        pump()

                    def make_fin_dve(hh, qq, psums, pctx):
                        box = {}

                        def fn():
                            craw = apool.tile(
                                [128, 512], F32, tag="ctx_raw", name=f"cr{hh}_{qq}"
                            )
                            nc.vector.tensor_copy(craw[:, :], pctx[:, :])
                            rf = apool.tile(
                                [1, 512], F32, tag="recipf", name=f"rf{hh}_{qq}"
                            )
                            nc.vector.reciprocal_approx_fast(rf[:, :], psums[:, :])
                            rc = apool.tile(
                                [1, 512], BF16, tag="recip", name=f"rc{hh}_{qq}"
                            )
                            nc.vector.tensor_copy(rc[:, :], rf[:, :])
                            box["craw"] = craw
                            box["rc"] = rc
                        return box, fn

                    box, fin_dve = make_fin_dve(h, q, ps_sums, ps_ctx)
                    if h == HG - 1 and q >= 2:
                        # tail chunks: flush the pipeline and emit the norm +
                        # collective input eagerly so the last AllGathers fire
                        # as soon as the data exists (they gate the final
                        # o_proj waves)
                        pump(force=True)
                        fin_dve()
                        emit_norm(h, q, box["craw"], box["rc"])
                        continue
                    pend.append((slot_box[0], 0, False, fin_dve))

                    def make_fin_pe(hh, qq, bx):
                        def fn():
                            emit_norm(hh, qq, bx["craw"], bx["rc"])
                        return fn

                    pend.append((slot_box[0], 4, True, make_fin_pe(h, q, box)))
            pump(force=True)

            # o_proj waves: wave h brings rows for global kt = 4r + h
            acc_sb = [
                accpool.tile([128, OC], F32, tag=f"acc{i}", name=f"acc{i}")
                for i in range(NST)
            ]

            def emit_half_wave(h, hf):
                cblk = []
                for r in range(TP):
                    t = cblkpool.tile(
                        [128, S // 2], BF16, tag=f"cblk{r}",
                        name=f"cb{h}_{hf}_{r}", bufs=2,
                    )
                    nc.sync.dma_start(
                        out=t[:, :], in_=cc_out[h][hf][r * 128 : (r + 1) * 128, :]
                    )
                    cblk.append(t)
                for j in range(NST // 2):
                    stile = hf * (NST // 2) + j
                    ps_po = psO.tile(
                        [128, OC], F32, tag="po", name=f"po{h}_{hf}_{j}"
                    )
                    for r in range(TP):
                        nc.tensor.matmul(
                            ps_po[:, :],
                            cblk[r][:, j * 128 : (j + 1) * 128],
                            wo_sb[:, 4 * r + h, :],
                            start=r == 0,
                            stop=r == TP - 1,
                        )
                    if h == 0:
                        nc.vector.tensor_copy(acc_sb[stile][:, :], ps_po[:, :])
                    else:
                        nc.vector.tensor_tensor(
                            acc_sb[stile][:, :], acc_sb[stile][:, :], ps_po[:, :],
                            op=ALU.add,
                        )
                    if h == HG - 1:
                        nc.sync.dma_start(
                            out=out[stile * 128 : (stile + 1) * 128, :],
                            in_=acc_sb[stile][:, :],
                        )

            for h in range(HG):
                for hf in range(2):
                    if h == HG - 1 and hf == 1:
                        continue  # final s-tiles arrive as quarter gathers
                    emit_half_wave(h, hf)
            # final quarter waves: head HG-1, s-tiles 8..15
            h = HG - 1
            for jq in (0, 1):
                cblk = []
                for r in range(TP):
                    t = cblkpool.tile(
                        [128, 512], BF16, tag=f"cblk{r}", name=f"cq{jq}_{r}", bufs=2
                    )
                    nc.sync.dma_start(
                        out=t[:, :], in_=cc_out_q[jq][r * 128 : (r + 1) * 128, :]
                    )
                    cblk.append(t)
                for j in range(4):
                    stile = 8 + jq * 4 + j
                    ps_po = psO.tile([128, OC], F32, tag="po", name=f"poq{jq}_{j}")
                    for r in range(TP):
                        nc.tensor.matmul(
                            ps_po[:, :],
                            cblk[r][:, j * 128 : (j + 1) * 128],
                            wo_sb[:, 4 * r + h, :],
                            start=r == 0,
                            stop=r == TP - 1,
                        )
                    nc.vector.tensor_tensor(
                        acc_sb[stile][:, :], acc_sb[stile][:, :], ps_po[:, :],
                        op=ALU.add,
                    )
                    nc.sync.dma_start(
                        out=out[stile * 128 : (stile + 1) * 128, :],
                        in_=acc_sb[stile][:, :],
                    )
            psO.release()
            ps2.release()
            cblkpool.release()
            accpool.release()
            ctxpool.release()
            pfpool.release()
            ppool.release()
            qkvpool.release()

    nc.compile()
    return nc


def _get_nc():
    if "nc" not in _CACHE:
        _CACHE["nc"] = _build()
    return _CACHE["nc"]


def _shard(hidden_states, position_ids, Wq, Wkv, Wo):
    """Host-side layout + bf16 conversion per core. No device work."""
    bf = ml_dtypes.bfloat16
    x = np.asarray(hidden_states, np.float32)
    pos = np.asarray(position_ids, np.int32)
    Wq = np.asarray(Wq, np.float32)
    Wkv = np.asarray(Wkv, np.float32)
    Wo = np.asarray(Wo, np.float32)

    def pkt(m):  # [free, HID] f32 -> [128, NKC, free] bf16
        # m.T is [HID, free]; row r = kt*128 + p
        t = np.ascontiguousarray(m.T).astype(bf)
        return np.ascontiguousarray(
            t.reshape(NKC, 128, m.shape[0]).transpose(1, 0, 2)
        )

    in_maps = []
    xs = [pkt(x[b]) for b in range(B)]
    wq_g = [pkt(Wq[g * OC : (g + 1) * OC]) for g in range(TP)]
    wo_g = [pkt(Wo[g * OC : (g + 1) * OC]) for g in range(TP)]
    wk_g = []
    wv_g = []
    for g in range(TP):
        krows = g * 2 * HD + 2 * np.arange(HD)
        wk_g.append(pkt(Wkv[krows]))
        wv_g.append(pkt(Wkv[krows + 1]))
    # rope tables per batch: [HD, S] with the half-dim frequencies tiled x2
    half = HD // 2
    invf = (1.0 / (ROPE_BASE ** (np.arange(half) / half))).astype(np.float32)
    invf2 = np.tile(invf, 2)[:, None]  # [HD, 1]
    sin_b, cos_b = [], []
    for b in range(B):
        ang = invf2 * pos[b][None, :].astype(np.float32)  # [HD, S]
        sin_b.append(np.ascontiguousarray(np.sin(ang)).astype(bf))
        cos_b.append(np.ascontiguousarray(np.cos(ang)).astype(bf))
    for c in range(N_CORES):
        b, g = c // TP, c % TP
        in_maps.append(
            {
                "xh": xs[b],
                "wqh": wq_g[g],
                "wkh": wk_g[g],
                "wvh": wv_g[g],
                "woh": wo_g[g],
                "sinh": sin_b[b],
                "cosh": cos_b[b],
            }
        )
    return in_maps


def run(hidden_states, position_ids, Wq, Wkv, Wo, trace=False):
    nc = _get_nc()
    in_maps = _shard(hidden_states, position_ids, Wq, Wkv, Wo)
    res = bass_utils.run_bass_kernel_spmd(
        nc, in_maps, core_ids=list(range(N_CORES)), trace=trace
    )
    out = np.empty((B, S, HID), np.float32)
    for c in range(N_CORES):
        b, g = c // TP, c % TP
        out[b][:, g * OC : (g + 1) * OC] = res.results[c]["out_slice"]
    return out, res


def kernel(hidden_states, position_ids, Wq, Wkv, Wo):
    out, _ = run(hidden_states, position_ids, Wq, Wkv, Wo, trace=False)
    return out
